# revision 29
# baseline (speedup 1.0000x reference)
"""Multi-head attention (B=2, T=2048, D=1024, H=16) on 8 NeuronCores.

Tensor-parallel over heads: 2 heads per core. Each core computes its
heads' QKV projection, causal attention, and a partial output
projection (its 128 columns of the concat dim); partials are summed on
the host.

Device dataflow is fully "transposed" (feature-major):
  - host supplies x^T [D, B*T]
  - qkv^T = W_slice @ x^T        (per-core W rows, pre-transposed host-side)
  - S^T[k,q] block = matmul(lhsT=K^T tile, rhs=Q^T tile), contraction dh=64
  - P^T = exp(S^T/8), causal mask applied only on diagonal-band blocks
  - O_aug^T [65, q] = V_aug.T @ P^T  with V_aug = [V | 1] so row 64
    accumulates the softmax denominator for free
  - normalize: DVE reciprocal straight from PSUM row 64, GPSIMD
    partition_broadcast to 64 rows, one DVE multiply
  - y^T partial [D, B*T] = (W_out slice)^T.T @ concatO^T

All matmul operands are fp16 (fp32 PSUM accumulation).

The kernel is PE-bound on real HW: each matmul costs ~(K_rows +
N_cols) PE cycles (weight load is not overlapped) plus ~45ns fixed.
Engines execute their instruction streams IN ORDER, so emission order
is the schedule:
  - per-batch pipelining: project(b), V transposes(b), attention(b) —
    ACT runs batch-0 exps while PE projects batch 1
  - attention is SOFTWARE-PIPELINED with depth 3 (LAG): the PV matmuls
    of S-unit u are emitted after the S matmuls of unit u+LAG, so PE
    never idles waiting for an exp; the 3-deep [128,2,512] PSUM ring
    holds the in-flight S tiles
  - phase C for batch 0 is interleaved into batch 1's attention as PE
    gap-filler; y stores are split per (batch, oi) row block
  - x^T loads as 32 [128,1024] contiguous-row DMAs on HWDGE (SP
    queue), kt-inner; weights go via SWDGE (Pool)
  - diag masks: one shared triangular mask (DVE multiply) for j=0,1;
    GPSIMD affine_select in place for j=2,3
"""

import contextlib
import sys

sys.path.insert(0, "/opt/trn_rl_repo")

import numpy as np

import concourse.bass as bass
import concourse.mybir as mybir
import concourse.tile as tile
from concourse import bacc
from concourse.masks import make_identity

B = 2
T = 2048
D = 1024
H = 16
DH = 64
N_CORES = 8
HPC = H // N_CORES          # heads per core = 2
F = HPC * DH                # per-core feature block = 128
TOK = B * T                 # 4096
P = 128                     # partitions
QB = 512                    # q block (free dim of S^T tiles)
KB = 128                    # k block (partition dim of S^T tiles)
NQB = T // QB               # 4 q blocks per instance
NKB = T // KB               # 16 k blocks per instance
NTT = TOK // QB             # 8 token tiles for projections
NKT = D // P                # 8 contraction tiles over D
NVA = B * HPC * NKB         # 64 V_aug tiles
LAG = 3                     # software pipeline depth in phase B

F32 = mybir.dt.float32
F32R = mybir.dt.float32r
F16 = mybir.dt.float16
EXP = mybir.ActivationFunctionType.Exp


def build_nc(loop_n: int = 1, phases: str = "ABC"):
    """loop_n > 1 wraps the whole kernel in an on-device For_i loop;
    phases selects a prefix ("A", "AB", "ABC") — both only for the
    timing harness."""
    nc = bacc.Bacc()

    xT = nc.dram_tensor("xT", [D, TOK], F16, kind="ExternalInput")
    wqkvT = nc.dram_tensor("wqkvT", [D, 3 * F], F16, kind="ExternalInput")
    woT = nc.dram_tensor("woT", [F, D], F16, kind="ExternalInput")
    yT = nc.dram_tensor("yT", [D, TOK], F16, kind="ExternalOutput")

    with tile.TileContext(nc) as tc:
        with (
            tc.tile_pool(name="const", bufs=1) as const,
            tc.tile_pool(name="big", bufs=1) as big,
            tc.tile_pool(name="psb", bufs=8) as psb,
            tc.tile_pool(name="small", bufs=4) as small,
            tc.tile_pool(name="ysb", bufs=4) as ysb,
            tc.tile_pool(name="ps2", bufs=3, space="PSUM") as ps2,
            tc.tile_pool(name="ps_o", bufs=2, space="PSUM") as ps_o,
        ):
            loop_ctx = (
                tc.For_i(0, loop_n, 1) if loop_n > 1 else contextlib.nullcontext()
            )
            with loop_ctx:
                build_body(nc, tc, const, big, psb, small, ysb,
                           ps2, ps_o, xT, wqkvT, woT, yT, phases)

    nc.compile()
    return nc


def build_body(nc, tc, const, big, psb, small, ysb,
               ps2, ps_o, xT, wqkvT, woT, yT, phases="ABC"):
    # ---- constants ----
    ident16 = const.tile([P, P], F16, tag="ident16")
    make_identity(nc, ident16[:])
    ones_row = const.tile([1, DH], F16, tag="ones_row")
    nc.gpsimd.memset(ones_row[:], 1.0)
    # shared triangular mask, replicated for both heads' halves:
    # tri2[k, half, i] = 1.0 if i >= k else 0
    tri2 = const.tile([P, 2, QB], F16, tag="tri2")
    nc.gpsimd.memset(tri2[:], 1.0)
    nc.gpsimd.affine_select(
        out=tri2[:],
        in_=tri2[:],
        compare_op=mybir.AluOpType.is_ge,
        fill=0.0,
        base=0,
        channel_multiplier=-1,
        pattern=[[0, 2], [1, QB]],
    )

    # ---- weights to SBUF (SWDGE so x loads head the HWDGE queue) ----
    w_sb = const.tile([P, NKT, 3 * F], F16, tag="w_sb")
    for kt in range(NKT):
        nc.gpsimd.dma_start(
            out=w_sb[:, kt, :], in_=wqkvT[kt * P : (kt + 1) * P, :]
        )
    wo_sb = const.tile([P, D], F16, tag="wo_sb")
    nc.gpsimd.dma_start(out=wo_sb[:], in_=woT[:, :])

    # ---- x^T to SBUF: 32 contiguous-row DMAs, kt-inner ----
    x_sb = [
        big.tile([P, TOK], F16, tag=f"x{kt}", name=f"x_sb{kt}")
        for kt in range(NKT)
    ]
    for tp in range(4):
        csl = np.s_[tp * 1024 : (tp + 1) * 1024]
        for kt in range(NKT):
            nc.sync.dma_start(
                out=x_sb[kt][:, csl], in_=xT[kt * P : (kt + 1) * P, csl]
            )

    # ---- per-512-token activation tiles ----
    QTs = [big.tile([P, QB], F16, tag=f"QT{i}", name=f"QTs{i}") for i in range(NTT)]
    KTs = [big.tile([P, QB], F16, tag=f"KT{i}", name=f"KTs{i}") for i in range(NTT)]
    VTs = [big.tile([P, QB], F16, tag=f"VT{i}", name=f"VTs{i}") for i in range(NTT)]
    COs = [big.tile([P, QB], F16, tag=f"CO{i}", name=f"COs{i}") for i in range(NTT)]
    # all V_aug tiles in one tensor: [128 tok, va_idx, 65] (col 64 = 1.0)
    vaug = big.tile([P, NVA, DH + 1], F16, tag="vaug")
    nc.gpsimd.memset(vaug[:, :, DH : DH + 1], 1.0)

    def emit_exp(p2ap, s2ap):
        if "a" in phases:
            nc.scalar.copy(p2ap, s2ap)
        elif "c" in phases:
            nc.vector.tensor_copy(p2ap, s2ap)
        else:
            nc.scalar.activation(p2ap, s2ap, EXP, scale=0.125)

    def emit_c_group(b, oi):
        """Phase C for one (batch, output-row-block): 4 matmuls, 2 copies,
        1 store of yT[oi*128:(oi+1)*128, b*T:(b+1)*T]."""
        y_sb = ysb.tile([P, T], F16, name=f"ysb_{b}_{oi}", tag="y")
        for q2 in range(NQB // 2):
            y2 = ps2.tile([P, 2, QB], F32, tag="s2", name=f"y2_{b}_{oi}_{q2}")
            for half in range(2):
                qb = 2 * q2 + half
                nc.tensor.matmul(
                    y2[:, half, :],
                    wo_sb[:, oi * P : (oi + 1) * P],
                    COs[b * NQB + qb][:],
                    start=True, stop=True,
                )
            dst = y_sb[:, q2 * 2 * QB : (q2 + 1) * 2 * QB]
            if (oi + q2) % 2 == 0:
                nc.scalar.copy(dst, y2[:])
            else:
                nc.vector.tensor_copy(dst, y2[:])
        nc.sync.dma_start(
            out=yT[oi * P : (oi + 1) * P, b * T : (b + 1) * T], in_=y_sb[:]
        )

    for b in range(B):
        # ---- phase A(b): qkv^T = W @ x^T for this batch's 4 token tiles ----
        for tt in range(b * NQB, (b + 1) * NQB):
            xsl = np.s_[tt * QB : (tt + 1) * QB]
            pr01 = ps2.tile([P, 2, QB], F32, tag="s2", name=f"pr01_{tt}")
            pr2 = ps_o.tile([P, QB], F32, tag="o", name=f"pr2_{tt}")
            for kt in range(NKT):
                st, sp = (kt == 0), (kt == NKT - 1)
                xs = x_sb[kt][:, xsl]
                nc.tensor.matmul(pr01[:, 0, :], w_sb[:, kt, 0:F], xs, start=st, stop=sp)
                nc.tensor.matmul(pr01[:, 1, :], w_sb[:, kt, F : 2 * F], xs, start=st, stop=sp)
                nc.tensor.matmul(pr2[:], w_sb[:, kt, 2 * F :], xs, start=st, stop=sp)
            nc.vector.tensor_copy(QTs[tt][:], pr01[:, 0, :])
            nc.vector.tensor_copy(KTs[tt][:], pr01[:, 1, :])
            nc.scalar.copy(VTs[tt][:], pr2[:])

        if phases == "A":
            continue

        # ---- phase A2(b): V^T -> V_aug (token-major), 4 transposes/copy ----
        for h in range(HPC):
            hsl = np.s_[h * DH : (h + 1) * DH]
            for kg in range(NKB // 4):
                tr = ps_o.tile([P, 4 * DH], F16, tag="o", name=f"tr_{b}_{h}_{kg}")
                for j in range(4):
                    ki = kg * 4 + j
                    src = VTs[b * NQB + ki // 4][
                        hsl, (ki % 4) * KB : (ki % 4 + 1) * KB
                    ]
                    nc.tensor.matmul(
                        tr[:, j * DH : (j + 1) * DH], src, ident16[hsl, hsl],
                        is_transpose=True, start=True, stop=True,
                    )
                va0 = (b * HPC + h) * NKB + kg * 4
                nc.vector.tensor_copy(vaug[:, va0 : va0 + 4, 0:DH], tr[:])

        # ---- phase B(b): software-pipelined attention ----
        # Build the unit list: each unit emits S matmuls + exp + mask at
        # issue time; its PV matmuls (+ trailing normalize) emit LAG
        # units later so PE never waits on ACT.
        units = []  # (emit_front, emit_back)
        o_tiles = {}  # (qi, h) -> psum tile, created at first PV
        pending_muls = []  # deferred normalize stage-2 closures

        def get_o(qi, h):
            key = (qi, h)
            if key not in o_tiles:
                o_tiles[key] = ps_o.tile(
                    [DH + 1, QB], F32, tag="o", name=f"o_{b}_{qi}_{h}"
                )
            return o_tiles[key]

        def normalize(qi, h):
            # Stage 1: copy O_aug out of PSUM (frees the o-ring slot at DVE
            # speed), reciprocal, and a GPSIMD broadcast. Pool does ONLY
            # broadcasts during B, so its ~2us q7 launch blocks nothing.
            # Stage 2 (the multiply, which waits on the broadcast) is
            # DEFERRED a few units so the in-order DVE stream never stalls.
            o_ps = o_tiles[(qi, h)]
            qrows = np.s_[h * DH : (h + 1) * DH]
            oc = small.tile([DH + 1, QB], F32, tag="oc")
            nc.vector.tensor_copy(oc[:], o_ps[:])
            r_sb = small.tile([1, QB], F32R, tag="rcp")
            with nc.allow_low_precision(reason="softmax recip bcast"):
                nc.vector.reciprocal(r_sb[:], oc[DH : DH + 1, :])
            rr_sb = small.tile([DH, QB], F32R, tag="rr")
            nc.gpsimd.partition_broadcast(rr_sb[:], r_sb[:], channels=DH)
            del o_tiles[(qi, h)]

            def stage2():
                nc.vector.tensor_mul(
                    COs[b * NQB + qi][qrows, :], oc[0:DH, :], rr_sb[:]
                )
            pending_muls.append(stage2)

        for qi in range(NQB):
            nblk = (qi + 1) * (QB // KB)
            ndiag = QB // KB
            npair_h = 2 * qi  # off-diag pairs per head
            qt = QTs[b * NQB + qi]

            def mk_diag(qi, j, nblk=nblk, ndiag=ndiag, qt=qt):
                ki = nblk - ndiag + j
                c0 = j * KB
                w = QB - c0
                kt_tile = KTs[b * NQB + ki // 4]
                ksl = np.s_[(ki % 4) * KB : (ki % 4 + 1) * KB]
                box = {}

                def front():
                    s2 = ps2.tile([P, 2, QB], F32, tag="s2", name=f"sd_{b}_{qi}_{j}")
                    p2 = psb.tile([P, 2, QB], F16, tag="p", name=f"pd_{b}_{qi}_{j}")
                    for h in range(HPC):
                        qrows = np.s_[h * DH : (h + 1) * DH]
                        nc.tensor.matmul(
                            s2[:, h, 0:w], kt_tile[qrows, ksl], qt[qrows, c0:QB],
                            start=True, stop=True,
                        )
                    emit_exp(p2[:, :, 0:w], s2[:, :, 0:w])
                    nc.vector.tensor_mul(
                        p2[:, :, 0:w], p2[:, :, 0:w], tri2[:, :, 0:w]
                    )
                    box["p2"] = p2

                def back():
                    p2 = box["p2"]
                    for h in range(HPC):
                        nc.tensor.matmul(
                            get_o(qi, h)[:, c0:QB],
                            vaug[:, (b * HPC + h) * NKB + ki, :],
                            p2[:, h, 0:w],
                            start=(j == 0),
                            stop=(qi == 0 and j == ndiag - 1),
                        )
                    if qi == 0 and j == ndiag - 1:
                        for h in range(HPC):
                            normalize(qi, h)

                return front, back

            def mk_pair(qi, h, k2, qt=qt, npair_h=npair_h):
                box = {}

                def front():
                    qrows = np.s_[h * DH : (h + 1) * DH]
                    s2 = ps2.tile([P, 2, QB], F32, tag="s2")
                    p2 = psb.tile([P, 2, QB], F16, tag="p")
                    for half in range(2):
                        ki = 2 * k2 + half
                        kt_tile = KTs[b * NQB + ki // 4]
                        nc.tensor.matmul(
                            s2[:, half, :],
                            kt_tile[qrows, (ki % 4) * KB : (ki % 4 + 1) * KB],
                            qt[qrows, :],
                            start=True, stop=True,
                        )
                    emit_exp(p2[:], s2[:])
                    box["p2"] = p2

                def back():
                    p2 = box["p2"]
                    last = k2 == npair_h - 1
                    for half in range(2):
                        ki = 2 * k2 + half
                        nc.tensor.matmul(
                            get_o(qi, h)[:],
                            vaug[:, (b * HPC + h) * NKB + ki, :],
                            p2[:, half, :],
                            start=False,
                            stop=(last and half == 1),
                        )
                    if last:
                        normalize(qi, h)

                return front, back

            # qi=0 has diag only; the diag PVs start each o_ps chain
            for j in range(ndiag):
                units.append(mk_diag(qi, j))
            for h in range(HPC):
                for k2 in range(npair_h):
                    units.append(mk_pair(qi, h, k2))

        # C(b-1) groups to interleave as PE gap-filler during this b's B
        fill = []
        if phases == "ABC" and b > 0:
            fill = [(b - 1, oi) for oi in range(D // P)]

        n_units = len(units)
        fill_every = max(1, n_units // (len(fill) + 1)) if fill else 0
        fi = 0
        for u in range(n_units + LAG):
            if u < n_units:
                units[u][0]()
            if u >= LAG:
                nm = len(pending_muls) - 2  # leave the 2 freshest pending
                for fn in pending_muls[:nm]:
                    fn()
                del pending_muls[:nm]
                units[u - LAG][1]()
            if fill and fi < len(fill) and u >= 8 and (u % fill_every) == 0:
                emit_c_group(*fill[fi])
                fi += 1
        for fn in pending_muls:
            fn()
        pending_muls.clear()
        while fill and fi < len(fill):
            emit_c_group(*fill[fi])
            fi += 1

    if phases == "A":
        for tt in range(NTT):
            for k, ts_ in enumerate((QTs, KTs, VTs)):
                nc.gpsimd.dma_start(
                    out=yT[k * P : (k + 1) * P, tt : tt + 1],
                    in_=ts_[tt][:, 0:1],
                )
        return

    if phases.startswith("AB") and phases != "ABC":
        for i in range(NTT):
            nc.gpsimd.dma_start(
                out=yT[0:P, i * QB : (i + 1) * QB], in_=COs[i][:]
            )
        return

    # ---- phase C for the last batch ----
    for oi in range(D // P):
        emit_c_group(B - 1, oi)


_NC = None


def get_nc():
    global _NC
    if _NC is None:
        _NC = build_nc()
    return _NC


def make_core_inputs(x, W_in, W_out):
    """Host-side sharding: per-core input maps."""
    xTh = np.ascontiguousarray(x.reshape(TOK, D).T).astype(np.float16)
    in_maps = []
    for c in range(N_CORES):
        rows = np.concatenate(
            [W_in[i * D + c * F : i * D + (c + 1) * F] for i in range(3)], axis=0
        )  # [384, 1024] = q|k|v rows for this core's 2 heads
        wqkvTh = np.ascontiguousarray(rows.T).astype(np.float16)
        woTh = np.ascontiguousarray(W_out[:, c * F : (c + 1) * F].T).astype(
            np.float16
        )
        in_maps.append({"xT": xTh, "wqkvT": wqkvTh, "woT": woTh})
    return in_maps


def kernel(x, W_in, W_out):
    from concourse.bass_utils import run_bass_kernel_spmd

    nc = get_nc()
    in_maps = make_core_inputs(
        np.asarray(x, dtype=np.float32),
        np.asarray(W_in, dtype=np.float32),
        np.asarray(W_out, dtype=np.float32),
    )
    res = run_bass_kernel_spmd(nc, in_maps, list(range(N_CORES)))
    y = np.zeros((D, TOK), dtype=np.float32)
    for r in res.results:
        y += r["yT"].astype(np.float32)
    return np.ascontiguousarray(y.T).reshape(B, T, D)
